# revision 20
# baseline (speedup 1.0000x reference)
"""Mixed-precision quantized linear (fp32/int8/int4/int2 weight groups) on 8 trn2 cores.

Strategy: tensor-parallel over output channels. Core k owns rows
[k*n_g/8, (k+1)*n_g/8) of every bit-group (128 + 384 + 512 + 256 = 1280
channels); x replicated. Host pre-transposes all weights to [K, N] with K
on partitions (original K order, no permutation).

v4: kt-outer just-in-time pipeline with dtype-packed DMA streams. Inputs
arrive as three interleaved tensors (bf16: x|w16 per K-tile, int8: q8|p2
per K-tile, fp8: p4|x8 pairs per 256-K block) so each DMA moves >=2.5KB
per partition row — small-descriptor DMA throughput collapse was the v3
bottleneck. Per K-tile the x tile is the stationary matmul operand and
weights stream. p4 runs as fp8e4 DoubleRow (host-unpacked int4 exact in
e4m3, x as a second e4m3 copy -> 2 K-tiles per matmul); q8/p2 are DVE/
GpSimd-cast int8->bf16. Bias rows fold in as K=1 matmuls issued first
(start=True). Epilogue: psum * per-channel scale (DVE for token block 0,
GpSimd for block 1, in parallel); output leaves as bf16, upcast on host.
Junk matmuls at the top warm the PE HAM clock gate while DMAs land.
"""

import numpy as np
import ml_dtypes

import concourse.bass as bass
import concourse.bacc as bacc
import concourse.mybir as mybir
import concourse.tile as tile
from concourse.bass_utils import run_bass_kernel_spmd

IN = 4096
OUT = 11008
N16, N8, N4, N2 = 1024, 3072, 4096, 2048
M = 256
NCORES = 8
C16, C8, C4, C2 = N16 // 8, N8 // 8, N4 // 8, N2 // 8  # 128, 384, 512, 256
NCH = C16 + C8 + C4 + C2  # 1280
KT = IN // 128  # 32 K-tiles
KTD = KT // 2  # 16 double K-tiles for fp8 DoubleRow

BWC = M + C16 + C8  # 768 bf16 cols per kt: [x 256 | w16 128 | q8*s8 384]
QPC = C2  # 256 int8 cols per kt: [p2]
F8C = C4 + M  # 768 fp8 cols per (ktd, pair): [p4 512 | x8 256]

BF16 = mybir.dt.bfloat16
F32 = mybir.dt.float32
I8 = mybir.dt.int8
F8 = mybir.dt.float8e4

Alu = mybir.AluOpType
DR = mybir.MatmulPerfMode.DoubleRow

NP_BF16 = ml_dtypes.bfloat16
NP_F8 = ml_dtypes.float8_e4m3

GROUP = 4  # kts per DMA group
N_WARM = 10  # junk matmuls to warm the HAM clock gate


def _build_nc():
    nc = bacc.Bacc()
    bw_d = nc.declare_dram_parameter("bw", [128, KT * BWC], BF16, isOutput=False)
    qp_d = nc.declare_dram_parameter("qp", [128, KT * QPC], I8, isOutput=False)
    f8_d = nc.declare_dram_parameter("f8", [128, KTD * 2 * F8C], F8, isOutput=False)
    sbc_d = nc.declare_dram_parameter("sbc", [128, C4 + C2], F32, isOutput=False)
    brow_d = nc.declare_dram_parameter("brow", [1, NCH], BF16, isOutput=False)
    out_d = nc.declare_dram_parameter("out", [M, NCH], BF16, isOutput=True)

    with tile.TileContext(nc) as tc:
        with (
            tc.tile_pool(name="big", bufs=1) as pool,
            tc.tile_pool(name="psum", bufs=1, space="PSUM") as ppool,
        ):
            bws = pool.tile([128, KT * BWC], BF16)
            qps = pool.tile([128, KT * QPC], I8)
            f8s = pool.tile([128, KTD * 2 * F8C], F8)
            p2b = pool.tile([128, KT * C2], BF16)
            sbcs = pool.tile([128, C4 + C2], F32)
            brs = pool.tile([1, NCH], BF16)
            brs2 = pool.tile([1, NCH], BF16)
            ones_b = pool.tile([1, 128], BF16)
            outs = pool.tile([128, 2 * NCH], BF16)
            warm_l = pool.tile([128, 128], BF16)
            warm_r = pool.tile([128, 512], BF16)

            ps = {
                (g, blk): ppool.tile([128, 512], F32, name=f"ps_{g}{blk}", tag=f"ps_{g}{blk}")
                for g in ("wq", "p4", "p2")
                for blk in (0, 1)
            }
            ps_junk = ppool.tile([128, 512], F32, name="ps_junk", tag="ps_junk")

            bwv = bws[:].rearrange("p (kt c) -> p kt c", c=BWC)
            qpv = qps[:].rearrange("p (kt c) -> p kt c", c=QPC)
            f8v = f8s[:].rearrange("p (ktd two c) -> p ktd two c", two=2, c=F8C)

            # --- warmup + constants (no DMA deps). Warm tiles memset on
            # GpSimd: it exits the entry barrier idle, so the junk matmuls
            # can start ~2us earlier than if they waited on DVE ---
            nc.gpsimd.memset(warm_l[:], 1.0)
            nc.gpsimd.memset(warm_r[:], 1.0)
            nc.vector.memset(ones_b[:], 1.0)
            for _ in range(N_WARM):
                nc.tensor.matmul(
                    ps_junk[:], warm_l[:], warm_r[:],
                    start=True, stop=True, skip_group_check=True,
                )

            # --- DMA issue order (sync engine is FIFO): brow first, then
            # kt groups just-in-time, sbc last (needed only at epilogue) ---
            nc.sync.dma_start(out=brs[:], in_=brow_d[:])
            for j in range(KT // GROUP):
                k0, k1 = j * GROUP, (j + 1) * GROUP
                d0, d1 = j * (GROUP // 2), (j + 1) * (GROUP // 2)
                nc.sync.dma_start(
                    out=bws[:, k0 * BWC : k1 * BWC], in_=bw_d[:, k0 * BWC : k1 * BWC]
                )
                nc.sync.dma_start(
                    out=qps[:, k0 * QPC : k1 * QPC], in_=qp_d[:, k0 * QPC : k1 * QPC]
                )
                # f8 stream rides the scalar engine's HWDGE ring so its
                # issue overlaps the sync engine's bw/qp issues
                nc.scalar.dma_start(
                    out=f8s[:, d0 * 2 * F8C : d1 * 2 * F8C],
                    in_=f8_d[:, d0 * 2 * F8C : d1 * 2 * F8C],
                )
            nc.sync.dma_start(out=sbcs[:], in_=sbc_d[:])

            # bias rows bounce through DVE so the K=1 bias matmuls have
            # all-DVE deps; they open each accumulation group (start=True)
            nc.vector.tensor_copy(brs2[:], brs[:])
            for blk in (0, 1):
                nc.tensor.matmul(
                    ps[("wq", blk)][:, : C16 + C8], ones_b[:1, :], brs2[:1, 0 : C16 + C8],
                    start=True, stop=False, skip_group_check=True,
                )
                nc.tensor.matmul(
                    ps[("p4", blk)][:, :C4], ones_b[:1, :],
                    brs2[:1, C16 + C8 : C16 + C8 + C4],
                    start=True, stop=False, skip_group_check=True,
                )
                nc.tensor.matmul(
                    ps[("p2", blk)][:, :C2], ones_b[:1, :], brs2[:1, C16 + C8 + C4 : NCH],
                    start=True, stop=False, skip_group_check=True,
                )

            # --- main kt loop ---
            for kt in range(KT):
                last = kt == KT - 1
                if kt % 2 == 0:
                    nc.vector.tensor_scalar(
                        p2b[:, kt * C2 : (kt + 2) * C2], qps[:, kt * C2 : (kt + 2) * C2],
                        0, None, op0=Alu.add,
                    )
                for blk in (0, 1):
                    xt_tile = bwv[:, kt, blk * 128 : blk * 128 + 128]
                    mms = [
                        ("wq", 0, C16 + C8, bwv[:, kt, M:BWC]),
                        ("p2", 0, C2, p2b[:, kt * C2 : (kt + 1) * C2]),
                    ]
                    if last:
                        mms = mms[::-1]
                    for g, c0, c1, rhs in mms:
                        nc.tensor.matmul(
                            ps[(g, blk)][:, c0:c1], xt_tile, rhs,
                            start=False, stop=last, skip_group_check=True,
                        )
                if kt % 2 == 0:
                    ktd = kt // 2
                    for blk in (0, 1):
                        nc.tensor.matmul(
                            ps[("p4", blk)][:, :C4],
                            f8v[:, ktd, :, C4 + blk * 128 : C4 + blk * 128 + 128],
                            f8v[:, ktd, :, 0:C4],
                            start=False, stop=ktd == KTD - 1,
                            perf_mode=DR, skip_group_check=True,
                        )

            # --- epilogue: p4/p2 psum * per-channel scale on DVE; wq scale
            # is pre-folded into the weights so its psum is a plain ACT
            # copy (runs parallel to DVE). p4 first (its accumulation ends
            # at kt 30); per-block order so each output DMA fires ASAP ---
            out_v = out_d[:].rearrange("(b p) n -> p b n", p=128)
            for blk in (0, 1):
                for g, c0, cw in (("p4", C16 + C8, C4), ("p2", C16 + C8 + C4, C2)):
                    nc.vector.scalar_tensor_tensor(
                        outs[:, blk * NCH + c0 : blk * NCH + c0 + cw],
                        ps[(g, blk)][:, :cw], 1.0,
                        sbcs[:, c0 - C16 - C8 : c0 - C16 - C8 + cw],
                        op0=Alu.mult, op1=Alu.mult,
                    )
                nc.scalar.copy(
                    outs[:, blk * NCH : blk * NCH + C16 + C8],
                    ps[("wq", blk)][:, : C16 + C8],
                )
                nc.sync.dma_start(
                    out=out_v[:, blk, :],
                    in_=outs[:, blk * NCH : (blk + 1) * NCH],
                )
    nc.finalize()
    return nc


def _unpack4(packed, K):
    """Host nibble unpack: int8-packed [N, K//2] -> signed int8 [N, K]."""
    u = np.asarray(packed).astype(np.int64) & 255
    low = u & 15
    high = (u >> 4) & 15
    full = np.stack([low, high], axis=-1).reshape(u.shape[0], -1)[:, :K]
    return np.where(full > 7, full - 16, full).astype(np.int8)


_CACHE = {}


def stage_inputs(**inputs):
    x = np.asarray(inputs["x"], dtype=np.float32)
    w16 = np.asarray(inputs["w16"], dtype=np.float32)
    b16 = np.asarray(inputs["b16"], dtype=np.float32)
    q8 = np.asarray(inputs["q8"])
    s8 = np.asarray(inputs["s8"], dtype=np.float32)
    b8 = np.asarray(inputs["b8"], dtype=np.float32)
    p4 = np.asarray(inputs["p4"])
    s4 = np.asarray(inputs["s4"], dtype=np.float32)
    b4 = np.asarray(inputs["b4"], dtype=np.float32)
    p2 = np.asarray(inputs["p2"])
    s2 = np.asarray(inputs["s2"], dtype=np.float32)
    b2 = np.asarray(inputs["b2"], dtype=np.float32)

    xT = np.ascontiguousarray(x.T)  # [4096, 256]
    xt3 = xT.astype(NP_BF16).reshape(KT, 128, M)  # [kt, p, t]
    x8r = xT.astype(NP_F8).reshape(KTD, 2, 128, M)  # [ktd, pair, p, t]

    v4 = _unpack4(p4, IN)  # [N4, IN] int8 in [-8, 7]
    v2 = _unpack4(p2, IN)  # [N2, IN] int8 in [-2, 1]

    in_maps = []
    cat_idxs = []
    for k in range(NCORES):
        w16k = w16[k * C16 : (k + 1) * C16]
        q8k = q8[k * C8 : (k + 1) * C8]
        v4k = v4[k * C4 : (k + 1) * C4]
        v2k = v2[k * C2 : (k + 1) * C2]
        s8k = s8[k * C8 : (k + 1) * C8, 0]
        s4k = s4[k * C4 : (k + 1) * C4, 0]
        s2k = s2[k * C2 : (k + 1) * C2, 0]

        # bf16 pack: [kt, p, x 256 | w16 128 | q8*s8 384] -> [128, KT*768]
        w163 = np.ascontiguousarray(w16k.T).astype(NP_BF16).reshape(KT, 128, C16)
        q8s = (q8k.astype(np.float32) * s8k[:, None]).T  # [IN, C8] pre-scaled
        q83 = np.ascontiguousarray(q8s).astype(NP_BF16).reshape(KT, 128, C8)
        bw = (
            np.concatenate([xt3, w163, q83], axis=2).transpose(1, 0, 2).reshape(128, -1)
        )
        # int8 pack: [kt, p, p2 256] -> [128, KT*256]
        qp = np.ascontiguousarray(v2k.T).reshape(KT, 128, C2).transpose(1, 0, 2).reshape(128, -1)
        # fp8 pack: [ktd, pair, p, p4 512 | x8 256] -> [128, KTD*1536]
        v4r = np.ascontiguousarray(v4k.T).astype(NP_F8).reshape(KTD, 2, 128, C4)
        f8 = (
            np.concatenate([v4r, x8r], axis=3)
            .transpose(2, 0, 1, 3)
            .reshape(128, -1)
        )

        srow = np.concatenate([s4k, s2k])
        sbc = np.ascontiguousarray(
            np.broadcast_to(srow[None, :], (128, C4 + C2))
        ).astype(np.float32)
        brow = (
            np.concatenate(
                [
                    b16[k * C16 : (k + 1) * C16],
                    b8[k * C8 : (k + 1) * C8],
                    b4[k * C4 : (k + 1) * C4] / s4k,
                    b2[k * C2 : (k + 1) * C2] / s2k,
                ]
            )
            .reshape(1, NCH)
            .astype(NP_BF16)
        )

        in_maps.append(
            {
                "bw": np.ascontiguousarray(bw),
                "qp": np.ascontiguousarray(qp),
                "f8": np.ascontiguousarray(f8),
                "sbc": sbc,
                "brow": brow,
            }
        )
        cat_idxs.append(
            np.concatenate(
                [
                    np.asarray(inputs["idx16"])[k * C16 : (k + 1) * C16],
                    np.asarray(inputs["idx8"])[k * C8 : (k + 1) * C8],
                    np.asarray(inputs["idx4"])[k * C4 : (k + 1) * C4],
                    np.asarray(inputs["idx2"])[k * C2 : (k + 1) * C2],
                ]
            )
        )
    return in_maps, cat_idxs


def kernel(**inputs):
    in_maps, cat_idxs = stage_inputs(**inputs)
    if "nc" not in _CACHE:
        _CACHE["nc"] = _build_nc()
    res = run_bass_kernel_spmd(_CACHE["nc"], in_maps, core_ids=list(range(NCORES)))
    _CACHE["last_res"] = res

    out = np.zeros((M, OUT), dtype=np.float32)
    for k in range(NCORES):
        out[:, cat_idxs[k]] = res.results[k]["out"].astype(np.float32)
    return out
